# revision 34
# baseline (speedup 1.0000x reference)
"""Trainium2 Bass kernel for nn_Block_Attention_3 (sparse_attention).

Contract: kernel(**inputs) takes FULL fp32 inputs (as in reference.setup_inputs())
and returns the FULL (4, 2304, 16, 16) fp32 output.

Strategy (zero-collective position sharding):
  The image is 16x16 = 4x4 grid of 4x4 patches. All cross-position coupling in
  the block stays within one (batch, patch-row) group: the incidence softmax is
  per-pixel over channels, the S-sum runs over image rows 4i..4i+3, and the
  attention is per-patch. So the 16 units (b, i) shard cleanly across 8 cores,
  2 units/core, with weights replicated. Replicating the folded conv weights
  costs less than the hint's contraction-shard + 4MB all-reduce round-trip
  through HBM, and needs no inter-core synchronization at all.

Per-core pipeline (single Bass program, SPMD over 8 cores):
  - inference BN is folded into conv weights/biases on the host; the final BN
    scale is folded into the V path; v-bias and out-BN scale ride the posA
    operand, so the V path has no bias matmul at all.
  - pixels are laid out patch-major: pix = u*64 + 16*jp + 4*ph + pw.
  - d,v convs accumulate in A-layout [pix, outch] (x chunk stationary);
    k,q convs in B-layout [outch, pix] (weight chunk stationary) so the
    scores matmul needs no transposes. Remaining biases enter as rank-1
    matmuls at the END of each PSUM accumulation group (warm PE clock).
  - weights stream in compute order d,q,k,v; the whole scores/softmax chain
    completes while wv is still streaming, leaving only the short V tail.
  - attention for all 8 patches runs as one batched 128x128 matmul pair; the
    block-diagonal -30000 mask is pre-accumulated into the scores PSUM via a
    single K=9 matmul (exact on-block zeros), then row-max subtraction and
    exp whose accum_out provides the softmax denominators.
"""
import os
import sys

sys.path.insert(0, "/opt/trn_rl_repo")

import numpy as np

EPS = 1e-5
D_IN, D, B, HW, P = 2048, 256, 4, 16, 4
NCHUNK = D_IN // 128  # 16
N_CORES = 8
MASK_NEG = 30000.0
COMPUTE_DT = os.environ.get("KERNEL_DT", "bfloat16")

_CACHE = {}

# rows aux layout (stored in compute dtype): [1, 1408]
_R_ONES = slice(0, 128)
_R_BD = slice(128, 384)            # d-conv bias (BN-folded)
_R_BETA = slice(384, 640)          # out-BN beta
_R_BKQ = slice(640, 1152)          # bk0|bk1|bq0|bq1 rows [1,128] each
ROWS_LEN = 1152


def _build_program(compute_dt_name: str):
    """Build (and compile to BIR) the single-core SPMD Bass program."""
    import concourse.mybir as mybir
    import concourse.tile as tile
    from concourse import bacc

    cdt = getattr(mybir.dt, compute_dt_name)
    f32 = mybir.dt.float32

    nc = bacc.Bacc("TRN2", target_bir_lowering=False, debug=False,
                   num_devices=N_CORES)

    x_d = nc.dram_tensor("x", [128, NCHUNK * 128], cdt, kind="ExternalInput")
    w_d = {name: nc.dram_tensor(name, [128, NCHUNK * 256 + (256 if name == "wq" else 0)],
                                cdt, kind="ExternalInput")
           for name in ("wd", "wv", "wk", "wq")}
    # combo: posA(+bv, so-scaled)[0:256] | R[256:264]
    combo_d = nc.dram_tensor("combo", [128, 264], f32, kind="ExternalInput")
    rows_d = nc.dram_tensor("rows", [1, ROWS_LEN], cdt, kind="ExternalInput")
    # mask9: K=9 rank decomposition of the additive block mask (compute dt)
    #   col 0:128 m-side (ones; blk indicators), 128:256 n-side (-M; M*blk)
    mask9_d = nc.dram_tensor("mask9", [9, 256], cdt, kind="ExternalInput")
    out_d = nc.dram_tensor("xloc", [128, 256], f32, kind="ExternalOutput")

    with tile.TileContext(nc) as tc:
        with (
            tc.tile_pool(name="big", bufs=1) as big,
            tc.tile_pool(name="small", bufs=1) as small,
            tc.tile_pool(name="ps", bufs=1, space="PSUM") as ps,
            tc.tile_pool(name="ps2", bufs=2, space="PSUM") as ps2,
        ):
            xt = big.tile([128, NCHUNK * 128], cdt, tag="xt")
            wt = {n: big.tile([128, NCHUNK * 256 + (256 if n == "wq" else 0)],
                              cdt, tag=n, name=n + "_t")
                  for n in ("wd", "wv", "wk", "wq")}
            combo = small.tile([128, 264], f32, tag="combo")
            rows = small.tile([1, ROWS_LEN], cdt, tag="rows")
            mask9 = small.tile([9, 256], cdt, tag="mask9")

            # ---- DMA loads (HWDGE). Stream order: x, wd, aux, then q,k,v in
            # outch-half-major packing so each half's conv (and everything
            # depending on it) completes while the next half still streams. ----
            q8 = (NCHUNK * 256) // 2
            q4 = (NCHUNK * 128) // 2
            for s in range(2):
                nc.sync.dma_start(xt[:, s * q4:(s + 1) * q4],
                                  x_d.ap()[:, s * q4:(s + 1) * q4])
                nc.sync.dma_start(wt["wd"][:, s * q8:(s + 1) * q8],
                                  w_d["wd"].ap()[:, s * q8:(s + 1) * q8])
            nc.sync.dma_start(rows[:], rows_d.ap())
            nc.sync.dma_start(mask9[:], mask9_d.ap())
            nc.sync.dma_start(combo[:], combo_d.ap())
            nc.sync.dma_start(wt["wq"][:, 0:q8], w_d["wq"].ap()[:, 0:q8])
            nc.sync.dma_start(wt["wq"][:, q8:2 * q8 + 256],
                              w_d["wq"].ap()[:, q8:2 * q8 + 256])
            qk = q8 // 2
            nc.sync.dma_start(wt["wk"][:, 0:q8], w_d["wk"].ap()[:, 0:q8])
            nc.sync.dma_start(wt["wk"][:, q8:q8 + qk],
                              w_d["wk"].ap()[:, q8:q8 + qk])
            nc.sync.dma_start(wt["wk"][:, q8 + qk:2 * q8],
                              w_d["wk"].ap()[:, q8 + qk:2 * q8])
            nc.sync.dma_start(wt["wv"][:, 0:q8], w_d["wv"].ap()[:, 0:q8])
            qv = q8 // 2
            nc.sync.dma_start(wt["wv"][:, q8:q8 + qv],
                              w_d["wv"].ap()[:, q8:q8 + qv])
            nc.sync.dma_start(wt["wv"][:, q8 + qv:2 * q8],
                              w_d["wv"].ap()[:, q8 + qv:2 * q8])

            posb = wt["wq"][:, NCHUNK * 256:NCHUNK * 256 + 256]
            posa = combo[:, 0:256]
            R_ap = combo[:, 256:264]
            ones_r = rows[0:1, _R_ONES]

            # ---- conv PSUM accumulators ----
            # ps pool: k0,k1,q0,q1,v0,v1 (one bank each); ps2 rotates
            # d -> S -> scores -> att0 -> att1 through two banks.
            d_ps = ps2.tile([128, 256], f32, tag="post", name="d_ps")
            kq_ps = [[ps.tile([128, 128], f32, tag=f"{n}{h}_ps", name=f"{n}{h}_ps")
                      for h in range(2)] for n in ("k", "q")]
            v_ps = [ps.tile([128, 128], f32, tag=f"v{g}_ps", name=f"v{g}_ps")
                    for g in range(2)]

            def a_conv(name, acc, brow):
                # A-layout [pix, outch], x chunk stationary; bias rank-1 last
                for c in range(NCHUNK):
                    nc.tensor.matmul(acc[:], xt[:, c * 128:(c + 1) * 128],
                                     wt[name][:, c * 256:(c + 1) * 256],
                                     start=(c == 0), stop=False)
                nc.tensor.matmul(acc[:], ones_r, brow, start=False, stop=True)

            def b_conv(name, wi, h):
                # B-layout [outch, pix], weight chunk stationary; bias last.
                # wk/wq are packed outch-half-major: col = h*2048 + c*128 + o.
                acc = kq_ps[wi][h]
                boff = _R_BKQ.start + (wi * 2 + h) * 128
                for c in range(NCHUNK):
                    nc.tensor.matmul(
                        acc[:],
                        wt[name][:, h * 2048 + c * 128:h * 2048 + (c + 1) * 128],
                        xt[:, c * 128:(c + 1) * 128],
                        start=(c == 0), stop=False)
                nc.tensor.matmul(acc[:], rows[0:1, boff:boff + 128], ones_r,
                                 start=False, stop=True)

            def v_conv(g):
                # A-layout half [pix, 128], wv packed outch-half-major
                for c in range(NCHUNK):
                    nc.tensor.matmul(
                        v_ps[g][:], xt[:, c * 128:(c + 1) * 128],
                        wt["wv"][:, g * 2048 + c * 128:g * 2048 + (c + 1) * 128],
                        start=(c == 0), stop=(c == NCHUNK - 1))

            a_conv("wd", d_ps, rows[0:1, _R_BD])
            b_conv("wq", 1, 0)
            b_conv("wq", 1, 1)
            b_conv("wk", 0, 0)
            b_conv("wk", 0, 1)

            # ---- d path: relu -> exp(accum) -> normalize -> S = incx.T @ R ----
            AF = mybir.ActivationFunctionType
            inc = small.tile([128, 256], f32, tag="inc")
            nc.scalar.activation(inc[:], d_ps[:], AF.Relu)
            einc = small.tile([128, 256], f32, tag="einc")
            dsum = small.tile([128, 1], f32, tag="dsum")
            nc.scalar.activation(einc[:], inc[:], AF.Exp, accum_out=dsum[:])
            dsuminv = small.tile([128, 1], f32, tag="dsuminv")
            nc.vector.reciprocal(dsuminv[:], dsum[:])
            incx = small.tile([128, 256], f32, tag="incx")
            nc.vector.tensor_scalar_mul(incx[:], einc[:], dsuminv[:, 0:1])
            s_ps = ps2.tile([128, 16], f32, tag="post", name="s_ps")
            sT = small.tile([128, 16], f32, tag="sT")
            for h in range(2):
                nc.tensor.matmul(s_ps[:, h * 8:(h + 1) * 8],
                                 incx[:, h * 128:(h + 1) * 128],
                                 R_ap, start=True, stop=True,
                                 skip_group_check=(h == 1))
            nc.vector.tensor_copy(sT[:], s_ps[:])

            # ---- scores PSUM: block mask first (one K=9 matmul), then Kp.T@J
            sc_ps = ps2.tile([128, 128], f32, tag="post", name="sc_ps")
            nc.tensor.matmul(sc_ps[:], mask9[:, 0:128], mask9[:, 128:256],
                             start=True, stop=False)

            # ---- k,q paths (B-layout): Kp = psum + pos ; J = psum*S + pos ----
            kp = [small.tile([128, 128], cdt, tag=f"kp{h}", name=f"kp{h}")
                  for h in range(2)]
            jp = [small.tile([128, 128], cdt, tag=f"jp{h}", name=f"jp{h}")
                  for h in range(2)]
            jtmp = [small.tile([128, 128], f32, tag=f"jtmp{h}", name=f"jtmp{h}")
                    for h in range(2)]
            for h in range(2):
                nc.vector.tensor_tensor(kp[h][:], kq_ps[0][h][:],
                                        posb[:, h * 128:(h + 1) * 128],
                                        op=mybir.AluOpType.add)
                s_bcast = sT[:, h * 8:(h + 1) * 8].unsqueeze(2).broadcast_to((128, 8, 16))
                q3d = kq_ps[1][h][:].rearrange("p (b w) -> p b w", b=8)
                j3d = jtmp[h][:].rearrange("p (b w) -> p b w", b=8)
                nc.vector.tensor_tensor(j3d, q3d, s_bcast, op=mybir.AluOpType.mult)
                nc.vector.tensor_tensor(jp[h][:], jtmp[h][:],
                                        posb[:, h * 128:(h + 1) * 128],
                                        op=mybir.AluOpType.add)
                nc.tensor.matmul(sc_ps[:], kp[h][:], jp[h][:],
                                 start=False, stop=(h == 1))

            # ---- att softmax over free dim (queries n) ----
            nmx = small.tile([128, 1], f32, tag="nmx")
            nc.vector.reduce_max(nmx[:], sc_ps[:], axis=mybir.AxisListType.X,
                                 negate=True)
            e_t = small.tile([128, 128], f32, tag="e_t")
            den = small.tile([128, 1], f32, tag="den")
            nc.scalar.activation(e_t[:], sc_ps[:], AF.Exp, bias=nmx[:, 0:1],
                                 accum_out=den[:])
            deninv = small.tile([128, 1], f32, tag="deninv")
            nc.vector.reciprocal(deninv[:], den[:])
            att = small.tile([128, 128], f32, tag="att")
            nc.vector.tensor_scalar_mul(att[:], e_t[:], deninv[:, 0:1])

            # ---- v halves (streamed last): per-half conv -> vpt -> V-matmul
            # -> copy -> DMA; half 0 completes while wv half 1 still streams --
            vpt = small.tile([128, 256], f32, tag="vpt")
            xloc = small.tile([128, 256], f32, tag="xloc")
            for g in range(2):
                gs = slice(g * 128, (g + 1) * 128)
                v_conv(g)
                nc.vector.tensor_tensor(vpt[:, gs], v_ps[g][:], posa[:, gs],
                                        op=mybir.AluOpType.add)
                att_ps = ps2.tile([128, 128], f32, tag="post", name=f"att_ps{g}")
                nc.tensor.matmul(att_ps[:], ones_r,
                                 rows[0:1, 384 + g * 128:384 + (g + 1) * 128],
                                 start=True, stop=False)
                nc.tensor.matmul(att_ps[:], att[:], vpt[:, gs], start=False, stop=True)
                nc.vector.tensor_copy(xloc[:, gs], att_ps[:])
                nc.sync.dma_start(out_d.ap()[:, gs], xloc[:, gs])

    nc.compile()
    return nc


def _fold_bn(w, b, g, beta, m, v):
    s = g / np.sqrt(v + EPS)
    return (w * s[:, None]).astype(np.float32), (s * (b - m) + beta).astype(np.float32)


def _np_dt(name):
    if name == "bfloat16":
        import ml_dtypes
        return ml_dtypes.bfloat16
    return np.float32


def _prep(inputs, np_dt):
    """Host-side prep: BN folds + per-core input maps."""
    inp = {k: np.asarray(v, dtype=np.float32) for k, v in inputs.items()}
    x, pos = inp["x"], inp["pos"]
    wk, bk = _fold_bn(inp["wk"], inp["bk"], inp["gk"], inp["betak"], inp["mk"], inp["vk"])
    wq, bq = _fold_bn(inp["wq"], inp["bq"], inp["gq"], inp["betaq"], inp["mq"], inp["vq"])
    wv, bv = _fold_bn(inp["wv"], inp["bv"], inp["gv"], inp["betav"], inp["mv"], inp["vv"])
    wd, bd = _fold_bn(inp["wd"], inp["bd"], inp["gd"], inp["betad"], inp["md"], inp["vd"])
    so = (inp["go"] / np.sqrt(inp["vo"] + EPS)).astype(np.float32)
    beta_o = (inp["beto"] - inp["mo"] * so).astype(np.float32)
    wv = wv * so[:, None]
    bv = bv * so  # folded into posA below

    def wpack_cmaj(w):  # chunk-major: [256 out, 2048 in] -> [128, (c,256)]
        wt = w.T.reshape(NCHUNK, 128, 256).transpose(1, 0, 2).reshape(128, -1)
        return np.ascontiguousarray(wt).astype(np_dt)

    def wpack_hmaj(w):  # outch-half-major: [256 out, 2048 in] -> [128, (h,c,128)]
        wt = w.T.reshape(NCHUNK, 128, 2, 128).transpose(1, 2, 0, 3).reshape(128, -1)
        return np.ascontiguousarray(wt).astype(np_dt)

    w_packed = {"wd": wpack_cmaj(wd), "wv": wpack_hmaj(wv),
                "wk": wpack_hmaj(wk)}
    wq_base = wpack_hmaj(wq)

    p_idx = np.arange(128)
    R = np.zeros((128, 8), np.float32)
    R[p_idx, (p_idx // 64) * 4 + (p_idx % 16) // 4] = 1.0
    pix_patch = (p_idx // 64) * 4 + (p_idx % 64) // 16
    blk_ind = (pix_patch[None, :] == np.arange(8)[:, None]).astype(np.float32)

    rows = np.zeros((1, ROWS_LEN), np.float32)
    rows[0, _R_ONES] = 1.0
    rows[0, _R_BD] = bd
    rows[0, _R_BETA] = beta_o
    rows[0, _R_BKQ] = np.concatenate([bk, bq])
    rows = rows.astype(np_dt)

    mask9 = np.zeros((9, 256), np.float32)
    mask9[0, 0:128] = 1.0
    mask9[0, 128:256] = -MASK_NEG
    mask9[1:9, 0:128] = blk_ind
    mask9[1:9, 128:256] = blk_ind * MASK_NEG
    mask9 = mask9.astype(np_dt)

    units = [(b, i) for b in range(B) for i in range(P)]
    in_maps = []
    for core in range(N_CORES):
        cu = units[2 * core:2 * core + 2]
        x_sb = np.empty((128, NCHUNK, 128), np.float32)
        pos_A = np.empty((128, 256), np.float32)
        posb_sb = np.empty((128, 256), np.float32)
        for u, (b, i) in enumerate(cu):
            # [c, ph, jp, pw] -> patch-major pixel (jp, ph, pw)
            xs = x[b, :, 4 * i:4 * i + 4, :].reshape(D_IN, 4, 4, 4)
            xs = xs.transpose(0, 2, 1, 3).reshape(D_IN, 64)
            x_sb[:, :, 64 * u:64 * u + 64] = xs.reshape(NCHUNK, 128, 64).transpose(1, 0, 2)
            ps_ = pos[b, :, 4 * i:4 * i + 4, :].reshape(D, 4, 4, 4).transpose(0, 2, 1, 3).reshape(D, 64)
            pos_A[64 * u:64 * u + 64, :] = ps_.T
            posb_sb[:, 64 * u:64 * u + 64] = ps_[0:128]
            posb_sb[:, 128 + 64 * u:128 + 64 * u + 64] = ps_[128:256]
        pos_A_sov = (pos_A * so[None, :] + bv[None, :]).astype(np.float32)
        wq_core = np.concatenate([wq_base, posb_sb.astype(np_dt)], axis=1)
        in_maps.append({
            "x": np.ascontiguousarray(x_sb.reshape(128, -1)).astype(np_dt),
            "wd": w_packed["wd"], "wv": w_packed["wv"],
            "wk": w_packed["wk"], "wq": np.ascontiguousarray(wq_core),
            "combo": np.concatenate([pos_A_sov, R], axis=1).astype(np.float32),
            "rows": rows, "mask9": mask9,
        })
    return in_maps, units


def _run_device(nc, in_maps):
    from concourse.bass_utils import run_bass_kernel_spmd
    return run_bass_kernel_spmd(nc, in_maps, list(range(N_CORES))).results


def _subproc_main(inp_path, out_path):
    import pickle
    with open(inp_path, "rb") as f:
        in_maps = pickle.load(f)
    nc = _build_program(COMPUTE_DT)
    res = _run_device(nc, in_maps)
    with open(out_path, "wb") as f:
        pickle.dump(res, f)


def _run_via_subprocess(in_maps):
    import pickle
    import subprocess
    import tempfile
    here = os.path.dirname(os.path.abspath(__file__))
    last = None
    for _ in range(2):
        with tempfile.TemporaryDirectory() as td:
            inp = os.path.join(td, "in.pkl")
            outp = os.path.join(td, "out.pkl")
            with open(inp, "wb") as f:
                pickle.dump(in_maps, f)
            code = (f"import sys; sys.path.insert(0, {here!r}); "
                    f"import kernel; kernel._subproc_main({inp!r}, {outp!r})")
            try:
                r = subprocess.run([sys.executable, "-c", code], timeout=1800)
                if r.returncode == 0 and os.path.exists(outp):
                    with open(outp, "rb") as f:
                        return pickle.load(f)
                last = RuntimeError(f"subprocess rc={r.returncode}")
            except Exception as e:  # noqa: BLE001
                last = e
    raise RuntimeError(f"device execution failed after retries: {last}")


def kernel(**inputs) -> np.ndarray:
    key = ("prog", COMPUTE_DT)
    if key not in _CACHE:
        _CACHE[key] = _build_program(COMPUTE_DT)
    nc = _CACHE[key]

    in_maps, units = _prep(inputs, _np_dt(COMPUTE_DT))
    try:
        results = _run_device(nc, in_maps)
    except Exception:
        # A crashed NEFF execution can poison this process's jax runtime
        # (NRT_EXEC_UNIT_UNRECOVERABLE); a fresh process recovers reliably.
        results = _run_via_subprocess(in_maps)

    x_loc = np.zeros((B, D, HW, HW), np.float32)
    for core in range(N_CORES):
        xl = results[core]["xloc"]  # [128 pix, 256 c]
        for u, (b, i) in enumerate(units[2 * core:2 * core + 2]):
            blk = xl[64 * u:64 * u + 64, :].reshape(4, 4, 4, D).transpose(3, 1, 0, 2)
            x_loc[b, :, 4 * i:4 * i + 4, :] = blk.reshape(D, 4, 16)
    return np.concatenate([np.asarray(inputs["x"], np.float32), x_loc], axis=1)
